# revision 26
# baseline (speedup 1.0000x reference)
"""Tensor-parallel MHA prefill kernel for 8 TRN2 NeuronCores.

Sharding: heads across cores (4 Q heads + 1 KV head per core).
Per core: QKV projection computed in transposed orientation
(out[qkv_dim, seq] = wqkv^T-stripes @ xT-stripes) so q/k arrive
pre-transposed for attention; RoPE applied via a host-side even/odd
row permutation of wq/wk (rotation partners become partition-offset
neighbours). Head-major causal attention with scores in
[kpos, qpos] orientation, exp on scalar engine, AV as
yT = v_aug^T @ exp(scores) accumulated in PSUM (ones column gives
softmax denominators; normalize via gpsimd partition_broadcast +
vector reciprocal). Per-head-pair AllToAll redistributes yT to
row-owners; pair-0 output-projection partials are parked in SBUF and
computed during the pair-1 AllToAll flight. Host only
slices/transposes/casts weights and the input, and concatenates the
8 output row-blocks.
"""
import os
import numpy as np
import ml_dtypes

N_CORES = 8
S = 2048
D = 2048
NH = 32
HD = 64
HPC = NH // N_CORES      # 4 q heads per core
QKV = HPC * HD + 2 * HD  # 384
SCALE = 1.0 / np.sqrt(HD)

ST = 128
NS = S // ST             # 16 seq tiles
DT = 128
ND = D // DT             # 16 d stripes
IC = 512                 # q-chunk width
NCH = S // IC            # 4 chunks
SROWS = S // N_CORES     # 256 output rows per core
HD1 = HD + 1             # 65 (v + ones column)

_CACHE = {}


def _build():
    from concourse import bacc
    import concourse.mybir as mybir
    from concourse.tile import TileContext
    from concourse.masks import make_identity

    dt = mybir.dt
    Exp = mybir.ActivationFunctionType.Exp
    nc = bacc.Bacc("TRN2", target_bir_lowering=False, debug=False,
                   num_devices=N_CORES)

    xT_d = nc.declare_dram_parameter("xT", [ND, 128, S], dt.bfloat16,
                                     isOutput=False)
    wqkvT = nc.declare_dram_parameter("wqkvT", [128, ND * QKV], dt.bfloat16,
                                      isOutput=False)
    woT = nc.declare_dram_parameter("woT", [128, ND * D], dt.bfloat16,
                                    isOutput=False)
    cosd = nc.declare_dram_parameter("cosd", [128, S], dt.bfloat16,
                                     isOutput=False)
    sind = nc.declare_dram_parameter("sind", [128, S], dt.bfloat16,
                                     isOutput=False)
    out = nc.declare_dram_parameter("out", [SROWS, D], dt.float32,
                                    isOutput=True)

    a2a_in = [nc.dram_tensor(f"a2a_in{p}", [N_CORES, 128, SROWS], dt.bfloat16)
              for p in range(2)]
    a2a_out = [nc.dram_tensor(f"a2a_out{p}", [N_CORES, 128, SROWS],
                              dt.bfloat16) for p in range(2)]

    IC2 = 2 * IC  # 1024: paired hh halves

    with TileContext(nc) as tc:
        const = tc.alloc_tile_pool(name="const", bufs=1)
        ident = const.tile([128, 128], dt.bfloat16, tag="ident")
        make_identity(nc, ident)
        dmask = const.tile([128, 128], dt.bfloat16, tag="dmask")
        nc.gpsimd.memset(dmask[:], 1.0)
        nc.gpsimd.affine_select(
            out=dmask[:], in_=dmask[:], compare_op=mybir.AluOpType.is_ge,
            fill=0.0, base=0, pattern=[[1, 128]], channel_multiplier=-1)

        pers = tc.alloc_tile_pool(name="pers", bufs=1)
        wq_sb = pers.tile([128, ND * QKV], dt.bfloat16, tag="wq")
        cs_sb = pers.tile([128, S], dt.bfloat16, tag="cs")
        sn_sb = pers.tile([128, S], dt.bfloat16, tag="sn")
        qT2 = [pers.tile([128, S], dt.bfloat16, tag=f"qT{p}", name=f"qT{p}")
               for p in range(2)]
        kk = pers.tile([128, S], dt.bfloat16, tag="kk")
        # per k-tile: cols [0:64] = v, [64:128] = ones (denominator block)
        v_aug = pers.tile([128, NS * 128], dt.bfloat16, tag="vaug")
        nc.gpsimd.memset(v_aug[:], 1.0)
        ys = [pers.tile([128, S], dt.bfloat16, tag=f"ys{p}", name=f"ys{p}")
              for p in range(2)]

        # input DMAs: wqkv first, then x stripes round-robin
        nc.sync.dma_start(out=wq_sb[:, 0:2 * QKV], in_=wqkvT[:, 0:2 * QKV])
        nc.scalar.dma_start(out=wq_sb[:, 2 * QKV:8 * QKV],
                            in_=wqkvT[:, 2 * QKV:8 * QKV])
        nc.gpsimd.dma_start(out=wq_sb[:, 8 * QKV:], in_=wqkvT[:, 8 * QKV:])
        nc.gpsimd.dma_start(out=cs_sb[:], in_=cosd[:])
        nc.gpsimd.dma_start(out=sn_sb[:], in_=sind[:])

        engs = [nc.sync, nc.scalar, nc.gpsimd]

        # ---------- phase 1: QKV projection + RoPE ----------------------
        with (
            tc.tile_pool(name="qkv_ps", bufs=6, space="PSUM") as qkv_ps,
            tc.tile_pool(name="tr_ps", bufs=2, space="PSUM") as tr_ps,
            tc.tile_pool(name="rtmp", bufs=3) as rtmp,
            tc.tile_pool(name="swp", bufs=2) as swp,
            tc.tile_pool(name="vst", bufs=2) as vst,
        ):
            xt_pool = tc.alloc_tile_pool(name="xt", bufs=1)
            xT = [xt_pool.tile([128, S], dt.bfloat16, tag=f"xT{i}",
                               name=f"xT{i}") for i in range(ND)]
            for i in range(ND):
                engs[i % 3].dma_start(out=xT[i][:], in_=xT_d[i])

            def rope_drain(ch, m, ps):
                sl = slice(ch * IC, (ch + 1) * IC)
                sw = swp.tile([128, IC], dt.bfloat16, tag="sw", name="sw")
                nrot = 128 if m < 2 else 64
                for b in range(nrot // 64):
                    nc.scalar.copy(sw[b * 64:b * 64 + 32, :],
                                   ps[b * 64 + 32:b * 64 + 64, :])
                    nc.vector.tensor_copy(sw[b * 64 + 32:b * 64 + 64, :],
                                          ps[b * 64:b * 64 + 32, :])
                t1 = rtmp.tile([128, IC], dt.float32, tag="t1", name="t1")
                t2 = rtmp.tile([128, IC], dt.float32, tag="t2", name="t2")
                nc.vector.tensor_mul(t1[0:nrot, :], ps[0:nrot, :],
                                     cs_sb[0:nrot, sl])
                nc.vector.tensor_mul(t2[0:nrot, :], sw[0:nrot, :],
                                     sn_sb[0:nrot, sl])
                if m < 2:
                    nc.vector.tensor_add(qT2[m][:, sl], t1[:], t2[:])
                else:
                    nc.vector.tensor_add(kk[0:HD, sl],
                                         t1[0:HD, :], t2[0:HD, :])
                    nc.vector.tensor_copy(kk[HD:128, sl], kk[0:HD, sl])
                    vs = vst.tile([HD, IC], dt.bfloat16, tag="vs",
                                  name="vs")
                    nc.scalar.copy(vs[:], ps[HD:128, :])
                    for t in range(4):
                        pt = tr_ps.tile([128, HD], dt.bfloat16,
                                        tag="tr", name="pt")
                        nc.tensor.transpose(
                            pt[:], vs[:, t * 128:(t + 1) * 128],
                            ident[0:HD, 0:HD])
                        st = 4 * ch + t
                        nc.vector.tensor_copy(
                            v_aug[:, st * 128:st * 128 + HD], pt[:])

            def qkv_wave(chunks, interleave):
                chains = [(ch, m) for ch in chunks for m in range(3)]
                tiles = {}
                for cm in chains:
                    tiles[cm] = qkv_ps.tile([128, IC], dt.float32,
                                            tag="qkv", name="qkv")
                if interleave:
                    for i in range(ND):
                        for (ch, m) in chains:
                            nc.tensor.matmul(
                                tiles[(ch, m)][:],
                                wq_sb[:, i * QKV + m * 128:
                                      i * QKV + (m + 1) * 128],
                                xT[i][:, ch * IC:(ch + 1) * IC],
                                start=(i == 0), stop=(i == ND - 1))
                    for (ch, m) in chains:
                        rope_drain(ch, m, tiles[(ch, m)])
                else:
                    for (ch, m) in chains:
                        for i in range(ND):
                            nc.tensor.matmul(
                                tiles[(ch, m)][:],
                                wq_sb[:, i * QKV + m * 128:
                                      i * QKV + (m + 1) * 128],
                                xT[i][:, ch * IC:(ch + 1) * IC],
                                start=(i == 0), stop=(i == ND - 1))
                        rope_drain(ch, m, tiles[(ch, m)])

            # wave A: 6 chains advance together as x stripes arrive
            qkv_wave((0, 1), interleave=True)
            # wave B: stripes all resident; chain-at-a-time
            qkv_wave((2, 3), interleave=False)
            xt_pool.release()

        # wo arrives during attention; host orders stripes even-mt first
        wo_pool = tc.alloc_tile_pool(name="wo_sb", bufs=1)
        wo_sb = wo_pool.tile([128, ND * D], dt.bfloat16, tag="wo")
        nc.sync.dma_start(out=wo_sb[:, 0:8 * D], in_=woT[:, 0:8 * D])
        nc.scalar.dma_start(out=wo_sb[:, 8 * D:], in_=woT[:, 8 * D:])
        ytf_pool = tc.alloc_tile_pool(name="ytf", bufs=1)

        # ---------- attention: pair-major, AV pipelined 2 jt behind -----
        with (
            tc.tile_pool(name="sc_ps", bufs=2, space="PSUM") as sc_ps,
            tc.tile_pool(name="yt_ps", bufs=2, space="PSUM") as yt_ps,
            tc.tile_pool(name="ets", bufs=1) as ets_pool,
            tc.tile_pool(name="den", bufs=2) as den_pool,
        ):

            def attn_unit(p, c):
                njt = 4 * c + 4
                ets = {}
                yt = yt_ps.tile([128, IC2], dt.float32, tag="yt", name="yt")

                def scores_exp(jt):
                    toff = jt - 4 * c
                    lo = max(toff, 0) * 128
                    ps_s = sc_ps.tile([128, IC2], dt.float32, tag="sc",
                                      name="sc")
                    for hh in range(2):
                        nc.tensor.matmul(
                            ps_s[:, hh * IC + lo:(hh + 1) * IC],
                            kk[hh * HD:hh * HD + HD,
                               jt * ST:(jt + 1) * ST],
                            qT2[p][hh * HD:hh * HD + HD,
                                   c * IC + lo:(c + 1) * IC],
                            start=True, stop=True,
                            tile_position=(hh * HD, 0))
                    et = ets_pool.tile([128, IC2], dt.bfloat16,
                                       tag=f"et{jt % 4}", name="et")
                    ets[jt] = et
                    src = ps_s[:].rearrange("q (h w) -> q h w",
                                            h=2)[:, :, lo:IC]
                    dst = et[:].rearrange("q (h w) -> q h w",
                                          h=2)[:, :, lo:IC]
                    nc.scalar.activation(dst, src, Exp, scale=float(SCALE))
                    if toff >= 0:
                        for hh in range(2):
                            nc.vector.tensor_mul(
                                et[:, hh * IC + lo:hh * IC + lo + 128],
                                et[:, hh * IC + lo:hh * IC + lo + 128],
                                dmask[:])

                def av(jt):
                    toff = jt - 4 * c
                    lo = max(toff, 0) * 128
                    for hh in range(2):
                        nc.tensor.matmul(
                            yt[:, hh * IC + lo:(hh + 1) * IC],
                            v_aug[:, jt * 128:(jt + 1) * 128],
                            ets[jt][:, hh * IC + lo:(hh + 1) * IC],
                            start=(jt == 0), stop=(jt == njt - 1))

                for jt in range(njt + 2):
                    if jt < njt:
                        scores_exp(jt)
                    if jt >= 2:
                        av(jt - 2)
                # rows 64:128 of yt are denominator copies (ones block)
                den_b = den_pool.tile([HD, IC2], dt.float32, tag="denb",
                                      name="denb")
                nc.vector.tensor_copy(den_b[:], yt[HD:128, :])
                rec_b = den_pool.tile([HD, IC2], dt.float32, tag="recb",
                                      name="recb")
                nc.vector.reciprocal_approx_fast(rec_b[:], den_b[:])
                for hh in range(2):
                    nc.vector.tensor_mul(
                        ys[p][hh * HD:(hh + 1) * HD, c * IC:(c + 1) * IC],
                        yt[0:HD, hh * IC:(hh + 1) * IC],
                        rec_b[:, hh * IC:(hh + 1) * IC])
                # stage this chunk's two destination slots right away
                for j in range(2):
                    r = 2 * c + j
                    (nc.sync if j == 0 else nc.scalar).dma_start(
                        out=a2a_in[p][r, :, :],
                        in_=ys[p][:, r * SROWS:(r + 1) * SROWS])

            def stage_pair(p):
                nc.gpsimd.collective_compute(
                    "AllToAll", mybir.AluOpType.bypass,
                    replica_groups=[list(range(N_CORES))],
                    ins=[a2a_in[p][:]], outs=[a2a_out[p][:]])

            ytf = {}

            def load_ytf(pr, dma_engs):
                for r in range(N_CORES):
                    ytf[(pr, r)] = ytf_pool.tile(
                        [128, SROWS], dt.bfloat16,
                        tag=f"ytf{pr}_{r}", name=f"ytf{pr}_{r}")
                    dma_engs[r % len(dma_engs)].dma_start(
                        out=ytf[(pr, r)][:], in_=a2a_out[pr][r, :, :])

            for c in range(NCH):
                attn_unit(0, c)
            stage_pair(0)
            for c in range(NCH):
                attn_unit(1, c)
            stage_pair(1)
            load_ytf(0, [nc.sync])
            load_ytf(1, [nc.scalar])


        # ---------- output projection -------------------------------
        with (
            tc.tile_pool(name="o_ps", bufs=8, space="PSUM") as o_ps,
            tc.tile_pool(name="park", bufs=1) as park_pool,
            tc.tile_pool(name="o_sb", bufs=3) as o_sb,
        ):
            park = {}
            # pass A: pair-0 partials (overlaps pair-1 AllToAll)
            for st in range(2):
                for nchk in range(4):
                    ps_o = o_ps.tile([128, 512], dt.float32, tag="o",
                                     name="o")
                    for r in range(N_CORES):
                        nc.tensor.matmul(
                            ps_o[:],
                            ytf[(0, r)][:, st * 128:(st + 1) * 128],
                            wo_sb[:, r * D + nchk * 512:
                                  r * D + (nchk + 1) * 512],
                            start=(r == 0), stop=(r == N_CORES - 1))
                    pk = park_pool.tile(
                        [128, 512], dt.float32,
                        tag=f"pk{st}{nchk}", name=f"pk{st}{nchk}")
                    park[(st, nchk)] = pk
                    nc.vector.tensor_copy(pk[:], ps_o[:])
            # pass B: pair-1 + parked partial -> out
            for st in range(2):
                for nchk in range(4):
                    ps_o = o_ps.tile([128, 512], dt.float32, tag="o",
                                     name="o")
                    for r in range(N_CORES):
                        nc.tensor.matmul(
                            ps_o[:],
                            ytf[(1, r)][:, st * 128:(st + 1) * 128],
                            wo_sb[:, (8 + r) * D + nchk * 512:
                                  (8 + r) * D + (nchk + 1) * 512],
                            start=(r == 0), stop=(r == N_CORES - 1))
                    ob = o_sb.tile([128, 512], dt.float32, tag="ob",
                                   name="ob")
                    nc.vector.tensor_add(ob[:], ps_o[:],
                                         park[(st, nchk)][:])
                    nc.sync.dma_start(
                        out=out[st * 128:(st + 1) * 128,
                                nchk * 512:(nchk + 1) * 512],
                        in_=ob[:])

        ytf_pool.release()
        wo_pool.release()
        pers.release()
        const.release()

    nc.compile()
    return nc


def _numpy_reference(x, freqs_cos, freqs_sin, input_pos, wq, wk, wv, wo,
                     k_cache, v_cache):
    B, S_, _ = x.shape
    NKV = 8
    n_rep = NH // NKV

    def rope(t, cos, sin):
        tr = t[..., 0::2]
        ti = t[..., 1::2]
        c = cos[None, :, None, :]
        s = sin[None, :, None, :]
        o = np.stack([tr * c - ti * s, tr * s + ti * c], axis=-1)
        return o.reshape(t.shape)

    q = (x @ wq.T).reshape(B, S_, NH, HD)
    k = (x @ wk.T).reshape(B, S_, NKV, HD)
    v = (x @ wv.T).reshape(B, S_, NKV, HD)
    q = rope(q, freqs_cos, freqs_sin).transpose(0, 2, 1, 3)
    k = rope(k, freqs_cos, freqs_sin).transpose(0, 2, 1, 3)
    v = v.transpose(0, 2, 1, 3)
    k_full = np.array(k_cache)
    v_full = np.array(v_cache)
    k_full[:, :, input_pos] = k
    v_full[:, :, input_pos] = v
    mask = np.tril(np.ones((k_full.shape[2], k_full.shape[2]), bool))[input_pos]
    k_rep = np.repeat(k_full, n_rep, axis=1)
    v_rep = np.repeat(v_full, n_rep, axis=1)
    sc = np.einsum("bhsd,bhtd->bhst", q, k_rep) * SCALE
    sc = np.where(mask[None, None], sc, -np.inf)
    sc = sc - sc.max(axis=-1, keepdims=True)
    e = np.exp(sc)
    attn = e / e.sum(axis=-1, keepdims=True)
    y = np.einsum("bhst,bhtd->bhsd", attn, v_rep)
    y = y.transpose(0, 2, 1, 3).reshape(B, S_, NH * HD)
    return (y @ wo.T).astype(np.float32)


def _rope_perm(nheads):
    """Row permutation: per 64-row head block -> [even dims, odd dims]."""
    idx = []
    for h in range(nheads):
        base = h * HD
        idx.extend(base + np.arange(0, HD, 2))
        idx.extend(base + np.arange(1, HD, 2))
    return np.array(idx)


def _stripe_fold(a, width):
    """[D, width] -> [128, ND*width] stripe-folded layout."""
    return np.ascontiguousarray(
        a.reshape(ND, 128, width).transpose(1, 0, 2).reshape(128, ND * width))


def kernel(x, freqs_cos, freqs_sin, input_pos, wq, wk, wv, wo,
           k_cache, v_cache):
    ipos = np.asarray(input_pos)
    if not np.array_equal(ipos, np.arange(S, dtype=ipos.dtype)):
        return _numpy_reference(np.asarray(x, np.float32),
                                np.asarray(freqs_cos), np.asarray(freqs_sin),
                                ipos, np.asarray(wq), np.asarray(wk),
                                np.asarray(wv), np.asarray(wo),
                                np.asarray(k_cache), np.asarray(v_cache))

    from concourse.bass_utils import run_bass_kernel_spmd

    if "nc" not in _CACHE:
        _CACHE["nc"] = _build()
    nc = _CACHE["nc"]

    bf16 = ml_dtypes.bfloat16
    x2 = np.asarray(x, np.float32)[0]
    xT = np.ascontiguousarray(x2.T.astype(bf16)).reshape(ND, 128, S)

    cos = np.asarray(freqs_cos, np.float32)   # [S, 32]
    sin = np.asarray(freqs_sin, np.float32)
    cosd = np.tile(cos.T, (4, 1)).astype(bf16)                # [128, S]
    sind = np.tile(np.concatenate([-sin.T, sin.T]), (2, 1)).astype(bf16)

    woTf = np.asarray(wo, np.float32).T.astype(bf16)          # [2048, 2048]
    # stripe order [0,2,...,14,1,3,...,15]: pair-0 mt blocks first
    wo_stripes = woTf.reshape(ND, 128, D)
    order = list(range(0, ND, 2)) + list(range(1, ND, 2))
    woT = np.ascontiguousarray(
        wo_stripes[order].transpose(1, 0, 2).reshape(128, ND * D))

    qperm = _rope_perm(HPC)
    kperm = _rope_perm(1)
    wq_f = np.asarray(wq, np.float32)
    wk_f = np.asarray(wk, np.float32)
    wv_f = np.asarray(wv, np.float32)

    in_maps = []
    for c in range(N_CORES):
        wq_c = wq_f[c * HPC * HD:(c + 1) * HPC * HD][qperm].T  # [2048, 256]
        wk_c = wk_f[c * HD:(c + 1) * HD][kperm].T              # [2048, 64]
        wv_c = wv_f[c * HD:(c + 1) * HD].T                     # [2048, 64]
        wqkv = np.concatenate([wq_c, wk_c, wv_c], axis=1).astype(bf16)
        in_maps.append({
            "xT": xT, "wqkvT": _stripe_fold(wqkv, QKV), "woT": woT,
            "cosd": cosd, "sind": sind,
        })

    res = run_bass_kernel_spmd(nc, in_maps, core_ids=list(range(N_CORES)),
                               trace=bool(os.environ.get("KERNEL_TRACE")))
    _CACHE["last_res"] = res
    rows = [res.results[c]["out"] for c in range(N_CORES)]
    return np.concatenate(rows, axis=0)[None].astype(np.float32)
